# revision 20
# baseline (speedup 1.0000x reference)
import os
import sys

import numpy as np

for _p in ("/opt/trn_rl_repo",):
    if _p not in sys.path and os.path.isdir(_p):
        sys.path.append(_p)

N = 1500
A = 64
STD = 0.3
PERSON_IDX = 2
INV2S2 = 1.0 / (2.0 * STD * STD)
SCALE = 2.0 * INV2S2

P = 128
NO = 1536            # padded objects (8 cores x 192)
NCORES = 8
OPC = NO // NCORES   # 192 objects per core
NOC = 256            # per-core object padding (2 tiles of 128)
NT = NOC // P        # 2 object tiles per core
KMAX = 24            # persons per device batch
GP = 8               # persons per matmul group
NG = KMAX // GP      # 4 groups
KK = 6 * GP + 1      # 49 contraction rows
NF = GP * A          # 512 free columns (person-in-group x action)

NEG = -1.0e9
TCLAMP = 16.0        # |t| clamp; any clamped pair has exp(-inv2s2*(16-2)^2) = 0
LNFLOOR = -20000.0   # floor for lnobj/SCALE row (fp16-safe, still exp -> 0)


def _mode():
    return os.environ.get("KERNEL_MM", "fp16hl")


def _hilo(a):
    hi = a.astype(np.float16)
    lo = (a - hi.astype(np.float32)).astype(np.float16)
    return hi, lo


def _obj_arrays(bbox, scores):
    best = scores.max(axis=1)
    idx = scores.argmax(axis=1)
    person = idx == PERSON_IDX
    obj = np.where(person, 0.0, best).astype(np.float32)

    w = bbox[:, 2] - bbox[:, 0]
    h = bbox[:, 3] - bbox[:, 1]
    cx = bbox[:, 0] + 0.5 * w
    cy = bbox[:, 1] + 0.5 * h

    cx_p = np.zeros(NO, np.float32); cx_p[:N] = cx
    cy_p = np.zeros(NO, np.float32); cy_p[:N] = cy
    lw_p = np.zeros(NO, np.float32); lw_p[:N] = np.log(w)
    lh_p = np.zeros(NO, np.float32); lh_p[:N] = np.log(h)
    lnobj_p = np.full(NO, NEG, np.float32)
    pos = obj > 0
    lnobj_p[:N] = np.where(pos, np.log(np.maximum(obj, 1e-38)), NEG)
    return person, best, w, h, cx, cy, cx_p, cy_p, lw_p, lh_p, lnobj_p


def _host_prep(hidx, best, w, h, cx, cy, obj_arr, target_mean, action_logits):
    """Build in_maps for one batch of <=KMAX persons (object-axis sharding)."""
    cx_p, cy_p, lw_p, lh_p, lnobj_p = obj_arr
    k = len(hidx)

    invw = np.ones(KMAX, np.float32); invw[:k] = 1.0 / w[hidx]
    invh = np.ones(KMAX, np.float32); invh[:k] = 1.0 / h[hidx]
    cxh = np.zeros(KMAX, np.float32); cxh[:k] = cx[hidx]
    cyh = np.zeros(KMAX, np.float32); cyh[:k] = cy[hidx]
    lwh = np.zeros(KMAX, np.float32); lwh[:k] = np.log(w[hidx])
    lhh = np.zeros(KMAX, np.float32); lhh[:k] = np.log(h[hidx])
    mu = np.zeros((KMAX, A, 4), np.float32); mu[:k] = target_mean[hidx]
    m2 = (mu * mu).sum(axis=-1)                               # [KMAX, A]
    lh_ = np.zeros((KMAX, A), np.float32)
    lh_[:k] = best[hidx][:, None] * action_logits[hidx]

    # person-side rhs [NG, KK, NF] block-diagonal (same for all cores)
    rhs = np.zeros((NG, KK, NF), np.float32)
    mug = mu.reshape(NG, GP, A, 4)
    m2g = m2.reshape(NG, GP, A)
    for j in range(GP):
        blk = slice(j * A, (j + 1) * A)
        for cc in range(4):
            rhs[:, cc * GP + j, blk] = mug[:, j, :, cc]
        rhs[:, 4 * GP + j, blk] = 1.0
        rhs[:, 5 * GP + j, blk] = -0.5 * m2g[:, j]
    rhs[:, 6 * GP, :] = 1.0

    lrow = lh_.reshape(NG, NF)
    if _mode() == "fp16hl":
        lrep = np.ascontiguousarray(
            np.broadcast_to(lrow[:, None, None, :], (NG, P, NT, NF))
        ).reshape(NG, P, NT * NF)
        bhi, blo = _hilo(rhs)
        rhs_hh = np.concatenate([bhi, bhi], axis=1)         # [NG, 2KK, NF]
        rhs_lo = blo                                        # [NG, KK, NF]
    else:
        lrep = np.ascontiguousarray(
            np.broadcast_to(lrow[:, None, :], (NG, P, NF))
        )
        rhs_hh = rhs_lo = None

    in_maps = []
    for c in range(NCORES):
        sl = slice(c * OPC, (c + 1) * OPC)
        cxo = np.zeros(NOC, np.float32); cxo[:OPC] = cx_p[sl]
        cyo = np.zeros(NOC, np.float32); cyo[:OPC] = cy_p[sl]
        lwo = np.zeros(NOC, np.float32); lwo[:OPC] = lw_p[sl]
        lho = np.zeros(NOC, np.float32); lho[:OPC] = lh_p[sl]
        lno = np.full(NOC, NEG, np.float32); lno[:OPC] = lnobj_p[sl]

        tx = cxo[None, :] * invw[:, None] - (cxh * invw)[:, None]   # [KMAX, NOC]
        ty = cyo[None, :] * invh[:, None] - (cyh * invh)[:, None]
        tw = lwo[None, :] - lwh[:, None]
        th = lho[None, :] - lhh[:, None]
        tx = np.clip(tx, -TCLAMP, TCLAMP)
        ty = np.clip(ty, -TCLAMP, TCLAMP)
        tw = np.clip(tw, -TCLAMP, TCLAMP)
        th = np.clip(th, -TCLAMP, TCLAMP)
        e2 = tx * tx + ty * ty + tw * tw + th * th

        lhsT = np.zeros((NG, KK, NOC), np.float32)
        g4 = lambda a: a.reshape(NG, GP, NOC)
        lhsT[:, 0:GP] = g4(tx)
        lhsT[:, GP:2 * GP] = g4(ty)
        lhsT[:, 2 * GP:3 * GP] = g4(tw)
        lhsT[:, 3 * GP:4 * GP] = g4(th)
        lhsT[:, 4 * GP:5 * GP] = g4(-0.5 * e2)
        lhsT[:, 5 * GP:6 * GP] = 1.0
        lhsT[:, 6 * GP] = np.maximum(lno / SCALE, LNFLOOR)

        if _mode() == "fp16hl":
            ahi, alo = _hilo(lhsT)
            blob = np.zeros((NG, 2 * KK, NOC + 2 * NF), np.float16)
            blob[:, :KK, :NOC] = ahi
            blob[:, KK:, :NOC] = alo
            blob[:, :, NOC:NOC + NF] = rhs_hh
            blob[:, :KK, NOC + NF:] = rhs_lo
            in_maps.append({"blob": blob, "lrep": lrep})
        else:
            in_maps.append({"lhst": lhsT, "rhs": rhs, "lrep": lrep})
    return in_maps


def _gather(results, k):
    parts = []
    for r in results:
        o = np.asarray(r["out"])
        if _mode() == "fp16hl":
            # [NG, P, NT*NF] -> persons x objects x actions
            o = o.reshape(NG, P, NT, GP, A)
            o = o.transpose(0, 3, 2, 1, 4).reshape(KMAX, NOC, A)
        else:
            o = o.reshape(NG, NT, P, GP, A)
            o = o.transpose(0, 3, 1, 2, 4).reshape(KMAX, NOC, A)
        parts.append(o[:k, :OPC, :])
    return np.concatenate(parts, axis=1)[:, :N, :]


_NC_CACHE = {}


def _build_nc():
    if "nc" in _NC_CACHE:
        return _NC_CACHE["nc"]
    import concourse.bacc as bacc
    import concourse.mybir as mybir
    from concourse.tile import TileContext

    f32 = mybir.dt.float32
    f16 = mybir.dt.float16
    mode = _mode()
    mmdt = mybir.dt.float32r if mode == "f32r" else mybir.dt.float32
    nc = bacc.Bacc()
    if mode == "fp16hl":
        WB = NOC + 2 * NF       # 1280 blob cols
        NW = NT * NF            # 1024 output cols
        blob_d = nc.dram_tensor(
            "blob", [NG, 2 * KK, WB], f16, kind="ExternalInput"
        )
        lrep_d = nc.dram_tensor("lrep", [NG, P, NW], f32, kind="ExternalInput")
        out_d = nc.dram_tensor("out", [NG, P, NW], f32, kind="ExternalOutput")

        with TileContext(nc) as tc:
            with (
                tc.tile_pool(name="wts", bufs=2) as wpool,
                tc.tile_pool(name="work", bufs=2) as work,
                tc.tile_pool(name="mmps", bufs=3, space="PSUM") as pspool,
            ):
                for g in range(NG):
                    blob = wpool.tile([2 * KK, WB], f16, tag="blob")
                    nc.sync.dma_start(blob[:], blob_d[g])
                    lrep = wpool.tile([P, NW], f32, tag="lrep")
                    nc.sync.dma_start(lrep[:], lrep_d[g])

                    ps = pspool.tile([P, NW], f32, tag="mm")
                    for t in range(NT):
                        csl = slice(t * NF, (t + 1) * NF)
                        psl = slice(t * P, (t + 1) * P)
                        nc.tensor.matmul(
                            ps[:, csl], blob[:, psl],
                            blob[:, NOC:NOC + NF],
                            start=True, stop=False,
                        )
                        nc.tensor.matmul(
                            ps[:, csl], blob[0:KK, psl],
                            blob[0:KK, NOC + NF:],
                            start=False, stop=True,
                        )
                    ex = work.tile([P, NW], f32, tag="ex")
                    nc.scalar.activation(
                        ex[:], ps[:], mybir.ActivationFunctionType.Exp,
                        scale=float(SCALE),
                    )
                    ot = work.tile([P, NW], f32, tag="ot")
                    nc.vector.tensor_mul(ot[:], ex[:], lrep[:])
                    nc.sync.dma_start(out_d[g], ot[:])
        nc.finalize()
        _NC_CACHE["nc"] = nc
        return nc

    lhst_d = nc.dram_tensor("lhst", [NG, KK, NOC], mmdt, kind="ExternalInput")
    rhs_d = nc.dram_tensor("rhs", [NG, KK, NF], mmdt, kind="ExternalInput")
    lrep_d = nc.dram_tensor("lrep", [NG, P, NF], f32, kind="ExternalInput")
    out_d = nc.dram_tensor("out", [NG, NT, P, NF], f32, kind="ExternalOutput")

    with TileContext(nc) as tc:
        with (
            tc.tile_pool(name="wts", bufs=2) as wpool,
            tc.tile_pool(name="work", bufs=4) as work,
            tc.tile_pool(name="mmps", bufs=4, space="PSUM") as pspool,
        ):
            for g in range(NG):
                lhsT = wpool.tile([KK, NOC], mmdt, tag="lhsT")
                nc.sync.dma_start(lhsT[:], lhst_d[g])
                rhs = wpool.tile([KK, NF], mmdt, tag="rhs")
                nc.sync.dma_start(rhs[:], rhs_d[g])
                lrep = wpool.tile([P, NF], f32, tag="lrep")
                nc.sync.dma_start(lrep[:], lrep_d[g])

                for t in range(NT):
                    ps = pspool.tile([P, NF], f32, tag="mm")
                    sl = slice(t * P, (t + 1) * P)
                    nc.tensor.matmul(
                        ps[:], lhsT[:, sl], rhs[:],
                        start=True, stop=True,
                    )
                    ex = work.tile([P, NF], f32, tag="ex")
                    nc.scalar.activation(
                        ex[:], ps[:], mybir.ActivationFunctionType.Exp,
                        scale=float(SCALE),
                    )
                    ot = work.tile([P, NF], f32, tag="ot")
                    nc.vector.tensor_mul(ot[:], ex[:], lrep[:])
                    nc.sync.dma_start(out_d[g, t], ot[:])
    nc.finalize()
    _NC_CACHE["nc"] = nc
    return nc


def _run_sim(in_maps):
    results = []
    for m in in_maps:
        lrep = m["lrep"]
        if _mode() == "fp16hl":
            out = np.zeros((NG, P, NT * NF), np.float32)
            for g in range(NG):
                b = m["blob"][g].astype(np.float32)         # [2KK, WB]
                a = b[:, :NOC]
                bh = b[:, NOC:NOC + NF]
                bl = b[:KK, NOC + NF:]
                mm = a.T @ bh + a[:KK, :].T @ bl            # [NOC, NF]
                ex = np.exp(np.minimum(SCALE * mm, 0.0).astype(np.float32))
                o = ex * lrep[g][:, :NF][0][None, :]        # [NOC, NF]
                out[g] = o.reshape(NT, P, NF).transpose(1, 0, 2).reshape(
                    P, NT * NF
                )
        else:
            out = np.zeros((NG, NT, P, NF), np.float32)
            for g in range(NG):
                mm = m["lhst"][g].T @ m["rhs"][g]
                ex = np.exp(np.minimum(SCALE * mm, 0.0).astype(np.float32))
                o = ex * lrep[g][0][None, :]
                out[g] = o.reshape(NT, P, NF)
        results.append({"out": out})
    return results


def kernel(action_logits, target_mean, bbox, scores):
    action_logits = np.asarray(action_logits, np.float32)
    target_mean = np.asarray(target_mean, np.float32)
    bbox = np.asarray(bbox, np.float32)
    scores = np.asarray(scores, np.float32)

    person, best, w, h, cx, cy, cx_p, cy_p, lw_p, lh_p, lnobj_p = _obj_arrays(
        bbox, scores
    )
    obj_arr = (cx_p, cy_p, lw_p, lh_p, lnobj_p)
    hidx_all = np.where(person)[0]

    full = np.zeros((N, N, A), np.float32)
    kernel.last_run = None
    for b0 in range(0, len(hidx_all), KMAX):
        hidx = hidx_all[b0:b0 + KMAX]
        in_maps = _host_prep(
            hidx, best, w, h, cx, cy, obj_arr, target_mean, action_logits
        )
        if os.environ.get("KERNEL_SIM") == "1":
            results = _run_sim(in_maps)
        else:
            from concourse.bass_utils import run_bass_kernel_spmd
            nc = _build_nc()
            kw = {}
            if os.environ.get("KERNEL_TRACE") == "1":
                kw = dict(trace=True, trace_cores=list(range(NCORES)))
            r = run_bass_kernel_spmd(
                nc, in_maps, core_ids=list(range(NCORES)), **kw
            )
            results = r.results
            kernel.last_run = r
        full[hidx] = _gather(results, len(hidx))
    return full


# revision 22
# speedup vs baseline: 1.1260x; 1.1260x over previous
import os
import sys

import numpy as np

for _p in ("/opt/trn_rl_repo",):
    if _p not in sys.path and os.path.isdir(_p):
        sys.path.append(_p)

N = 1500
A = 64
STD = 0.3
PERSON_IDX = 2
INV2S2 = 1.0 / (2.0 * STD * STD)
SCALE = 2.0 * INV2S2

P = 128
NO = 1536            # padded objects (8 cores x 192)
NCORES = 8
OPC = NO // NCORES   # 192 objects per core
NOC = 256            # per-core object padding (2 tiles of 128)
NT = NOC // P        # 2 object tiles per core
KMAX = 24            # persons per device batch
GP = 8               # persons per matmul group
NG = KMAX // GP      # 4 groups
KK = 6 * GP + 1      # 49 contraction rows
NF = GP * A          # 512 free columns (person-in-group x action)

NEG = -1.0e9
TCLAMP = 16.0        # |t| clamp; any clamped pair has exp(-inv2s2*(16-2)^2) = 0
LNFLOOR = -20000.0   # floor for lnobj/SCALE row (fp16-safe, still exp -> 0)


def _mode():
    return os.environ.get("KERNEL_MM", "fp16hl")


def _hilo(a):
    hi = a.astype(np.float16)
    lo = (a - hi.astype(np.float32)).astype(np.float16)
    return hi, lo


def _obj_arrays(bbox, scores):
    best = scores.max(axis=1)
    idx = scores.argmax(axis=1)
    person = idx == PERSON_IDX
    obj = np.where(person, 0.0, best).astype(np.float32)

    w = bbox[:, 2] - bbox[:, 0]
    h = bbox[:, 3] - bbox[:, 1]
    cx = bbox[:, 0] + 0.5 * w
    cy = bbox[:, 1] + 0.5 * h

    cx_p = np.zeros(NO, np.float32); cx_p[:N] = cx
    cy_p = np.zeros(NO, np.float32); cy_p[:N] = cy
    lw_p = np.zeros(NO, np.float32); lw_p[:N] = np.log(w)
    lh_p = np.zeros(NO, np.float32); lh_p[:N] = np.log(h)
    lnobj_p = np.full(NO, NEG, np.float32)
    pos = obj > 0
    lnobj_p[:N] = np.where(pos, np.log(np.maximum(obj, 1e-38)), NEG)
    return person, best, w, h, cx, cy, cx_p, cy_p, lw_p, lh_p, lnobj_p


def _host_prep(hidx, best, w, h, cx, cy, obj_arr, target_mean, action_logits):
    """Build in_maps for one batch of <=KMAX persons (object-axis sharding)."""
    cx_p, cy_p, lw_p, lh_p, lnobj_p = obj_arr
    k = len(hidx)

    invw = np.ones(KMAX, np.float32); invw[:k] = 1.0 / w[hidx]
    invh = np.ones(KMAX, np.float32); invh[:k] = 1.0 / h[hidx]
    cxh = np.zeros(KMAX, np.float32); cxh[:k] = cx[hidx]
    cyh = np.zeros(KMAX, np.float32); cyh[:k] = cy[hidx]
    lwh = np.zeros(KMAX, np.float32); lwh[:k] = np.log(w[hidx])
    lhh = np.zeros(KMAX, np.float32); lhh[:k] = np.log(h[hidx])
    mu = np.zeros((KMAX, A, 4), np.float32); mu[:k] = target_mean[hidx]
    m2 = (mu * mu).sum(axis=-1)                               # [KMAX, A]
    lh_ = np.zeros((KMAX, A), np.float32)
    lh_[:k] = best[hidx][:, None] * action_logits[hidx]

    # person-side rhs [NG, KK, NF] block-diagonal (same for all cores)
    rhs = np.zeros((NG, KK, NF), np.float32)
    mug = mu.reshape(NG, GP, A, 4)
    m2g = m2.reshape(NG, GP, A)
    for j in range(GP):
        blk = slice(j * A, (j + 1) * A)
        for cc in range(4):
            rhs[:, cc * GP + j, blk] = mug[:, j, :, cc]
        rhs[:, 4 * GP + j, blk] = 1.0
        rhs[:, 5 * GP + j, blk] = -0.5 * m2g[:, j]
    rhs[:, 6 * GP, :] = 1.0

    lrow = lh_.reshape(NG, NF)
    if _mode() == "fp16hl":
        lrep = np.ascontiguousarray(
            np.broadcast_to(lrow[:, None, :], (NG, P, NF))
        )
        bhi, blo = _hilo(rhs)
        rhs_hh = np.concatenate([bhi, bhi], axis=1)         # [NG, 2KK, NF]
        rhs_lo = blo                                        # [NG, KK, NF]
    else:
        lrep = np.ascontiguousarray(
            np.broadcast_to(lrow[:, None, :], (NG, P, NF))
        )
        rhs_hh = rhs_lo = None

    in_maps = []
    for c in range(NCORES):
        sl = slice(c * OPC, (c + 1) * OPC)
        cxo = np.zeros(NOC, np.float32); cxo[:OPC] = cx_p[sl]
        cyo = np.zeros(NOC, np.float32); cyo[:OPC] = cy_p[sl]
        lwo = np.zeros(NOC, np.float32); lwo[:OPC] = lw_p[sl]
        lho = np.zeros(NOC, np.float32); lho[:OPC] = lh_p[sl]
        lno = np.full(NOC, NEG, np.float32); lno[:OPC] = lnobj_p[sl]

        tx = cxo[None, :] * invw[:, None] - (cxh * invw)[:, None]   # [KMAX, NOC]
        ty = cyo[None, :] * invh[:, None] - (cyh * invh)[:, None]
        tw = lwo[None, :] - lwh[:, None]
        th = lho[None, :] - lhh[:, None]
        tx = np.clip(tx, -TCLAMP, TCLAMP)
        ty = np.clip(ty, -TCLAMP, TCLAMP)
        tw = np.clip(tw, -TCLAMP, TCLAMP)
        th = np.clip(th, -TCLAMP, TCLAMP)
        e2 = tx * tx + ty * ty + tw * tw + th * th

        lhsT = np.zeros((NG, KK, NOC), np.float32)
        g4 = lambda a: a.reshape(NG, GP, NOC)
        lhsT[:, 0:GP] = g4(tx)
        lhsT[:, GP:2 * GP] = g4(ty)
        lhsT[:, 2 * GP:3 * GP] = g4(tw)
        lhsT[:, 3 * GP:4 * GP] = g4(th)
        lhsT[:, 4 * GP:5 * GP] = g4(-0.5 * e2)
        lhsT[:, 5 * GP:6 * GP] = 1.0
        lhsT[:, 6 * GP] = np.maximum(lno / SCALE, LNFLOOR)

        if _mode() == "fp16hl":
            ahi, alo = _hilo(lhsT)
            blob = np.zeros((NG, 2 * KK, NOC + 2 * NF), np.float16)
            blob[:, :KK, :NOC] = ahi
            blob[:, KK:, :NOC] = alo
            blob[:, :, NOC:NOC + NF] = rhs_hh
            blob[:, :KK, NOC + NF:] = rhs_lo
            in_maps.append({"blob": blob, "lrep": lrep})
        else:
            in_maps.append({"lhst": lhsT, "rhs": rhs, "lrep": lrep})
    return in_maps


def _gather(results, k):
    parts = []
    for r in results:
        o = np.asarray(r["out"])
        if _mode() == "fp16hl":
            # [NG, P, NT*NF] -> persons x objects x actions
            o = o.reshape(NG, P, NT, GP, A)
            o = o.transpose(0, 3, 2, 1, 4).reshape(KMAX, NOC, A)
        else:
            o = o.reshape(NG, NT, P, GP, A)
            o = o.transpose(0, 3, 1, 2, 4).reshape(KMAX, NOC, A)
        parts.append(o[:k, :OPC, :])
    return np.concatenate(parts, axis=1)[:, :N, :]


_NC_CACHE = {}


def _build_nc():
    if "nc" in _NC_CACHE:
        return _NC_CACHE["nc"]
    import concourse.bacc as bacc
    import concourse.mybir as mybir
    from concourse.tile import TileContext

    f32 = mybir.dt.float32
    f16 = mybir.dt.float16
    mode = _mode()
    mmdt = mybir.dt.float32r if mode == "f32r" else mybir.dt.float32
    nc = bacc.Bacc()
    if mode == "fp16hl":
        WB = NOC + 2 * NF       # 1280 blob cols
        NW = NT * NF            # 1024 output cols
        blob_d = nc.dram_tensor(
            "blob", [NG, 2 * KK, WB], f16, kind="ExternalInput"
        )
        lrep_d = nc.dram_tensor("lrep", [NG, P, NF], f32, kind="ExternalInput")
        out_d = nc.dram_tensor("out", [NG, P, NW], f32, kind="ExternalOutput")

        with TileContext(nc) as tc:
            with (
                tc.tile_pool(name="wts", bufs=3) as wpool,
                tc.tile_pool(name="work", bufs=3) as work,
                tc.tile_pool(name="mmps", bufs=3, space="PSUM") as pspool,
            ):
                for g in range(NG):
                    blob = wpool.tile([2 * KK, WB], f16, tag="blob")
                    nc.sync.dma_start(blob[:], blob_d[g])
                    lrep = wpool.tile([P, NF], f32, tag="lrep")
                    nc.sync.dma_start(lrep[:], lrep_d[g])

                    ps = pspool.tile([P, NW], f32, tag="mm")
                    for t in range(NT):
                        csl = slice(t * NF, (t + 1) * NF)
                        psl = slice(t * P, (t + 1) * P)
                        nc.tensor.matmul(
                            ps[:, csl], blob[:, psl],
                            blob[:, NOC:NOC + NF],
                            start=True, stop=False,
                        )
                        nc.tensor.matmul(
                            ps[:, csl], blob[0:KK, psl],
                            blob[0:KK, NOC + NF:],
                            start=False, stop=True,
                        )
                        ex = work.tile([P, NF], f32, tag="ex")
                        nc.scalar.activation(
                            ex[:], ps[:, csl], mybir.ActivationFunctionType.Exp,
                            scale=float(SCALE),
                        )
                        ot = work.tile([P, NF], f32, tag="ot")
                        nc.vector.tensor_mul(ot[:], ex[:], lrep[:])
                        nc.sync.dma_start(out_d[g][:, csl], ot[:])
        nc.finalize()
        _NC_CACHE["nc"] = nc
        return nc

    lhst_d = nc.dram_tensor("lhst", [NG, KK, NOC], mmdt, kind="ExternalInput")
    rhs_d = nc.dram_tensor("rhs", [NG, KK, NF], mmdt, kind="ExternalInput")
    lrep_d = nc.dram_tensor("lrep", [NG, P, NF], f32, kind="ExternalInput")
    out_d = nc.dram_tensor("out", [NG, NT, P, NF], f32, kind="ExternalOutput")

    with TileContext(nc) as tc:
        with (
            tc.tile_pool(name="wts", bufs=2) as wpool,
            tc.tile_pool(name="work", bufs=4) as work,
            tc.tile_pool(name="mmps", bufs=4, space="PSUM") as pspool,
        ):
            for g in range(NG):
                lhsT = wpool.tile([KK, NOC], mmdt, tag="lhsT")
                nc.sync.dma_start(lhsT[:], lhst_d[g])
                rhs = wpool.tile([KK, NF], mmdt, tag="rhs")
                nc.sync.dma_start(rhs[:], rhs_d[g])
                lrep = wpool.tile([P, NF], f32, tag="lrep")
                nc.sync.dma_start(lrep[:], lrep_d[g])

                for t in range(NT):
                    ps = pspool.tile([P, NF], f32, tag="mm")
                    sl = slice(t * P, (t + 1) * P)
                    nc.tensor.matmul(
                        ps[:], lhsT[:, sl], rhs[:],
                        start=True, stop=True,
                    )
                    ex = work.tile([P, NF], f32, tag="ex")
                    nc.scalar.activation(
                        ex[:], ps[:], mybir.ActivationFunctionType.Exp,
                        scale=float(SCALE),
                    )
                    ot = work.tile([P, NF], f32, tag="ot")
                    nc.vector.tensor_mul(ot[:], ex[:], lrep[:])
                    nc.sync.dma_start(out_d[g, t], ot[:])
    nc.finalize()
    _NC_CACHE["nc"] = nc
    return nc


def _run_sim(in_maps):
    results = []
    for m in in_maps:
        lrep = m["lrep"]
        if _mode() == "fp16hl":
            out = np.zeros((NG, P, NT * NF), np.float32)
            for g in range(NG):
                b = m["blob"][g].astype(np.float32)         # [2KK, WB]
                a = b[:, :NOC]
                bh = b[:, NOC:NOC + NF]
                bl = b[:KK, NOC + NF:]
                mm = a.T @ bh + a[:KK, :].T @ bl            # [NOC, NF]
                ex = np.exp(np.minimum(SCALE * mm, 0.0).astype(np.float32))
                o = ex * lrep[g][:, :NF][0][None, :]        # [NOC, NF]
                out[g] = o.reshape(NT, P, NF).transpose(1, 0, 2).reshape(
                    P, NT * NF
                )
        else:
            out = np.zeros((NG, NT, P, NF), np.float32)
            for g in range(NG):
                mm = m["lhst"][g].T @ m["rhs"][g]
                ex = np.exp(np.minimum(SCALE * mm, 0.0).astype(np.float32))
                o = ex * lrep[g][0][None, :]
                out[g] = o.reshape(NT, P, NF)
        results.append({"out": out})
    return results


def kernel(action_logits, target_mean, bbox, scores):
    action_logits = np.asarray(action_logits, np.float32)
    target_mean = np.asarray(target_mean, np.float32)
    bbox = np.asarray(bbox, np.float32)
    scores = np.asarray(scores, np.float32)

    person, best, w, h, cx, cy, cx_p, cy_p, lw_p, lh_p, lnobj_p = _obj_arrays(
        bbox, scores
    )
    obj_arr = (cx_p, cy_p, lw_p, lh_p, lnobj_p)
    hidx_all = np.where(person)[0]

    full = np.zeros((N, N, A), np.float32)
    kernel.last_run = None
    for b0 in range(0, len(hidx_all), KMAX):
        hidx = hidx_all[b0:b0 + KMAX]
        in_maps = _host_prep(
            hidx, best, w, h, cx, cy, obj_arr, target_mean, action_logits
        )
        if os.environ.get("KERNEL_SIM") == "1":
            results = _run_sim(in_maps)
        else:
            from concourse.bass_utils import run_bass_kernel_spmd
            nc = _build_nc()
            kw = {}
            if os.environ.get("KERNEL_TRACE") == "1":
                kw = dict(trace=True, trace_cores=list(range(NCORES)))
            r = run_bass_kernel_spmd(
                nc, in_maps, core_ids=list(range(NCORES)), **kw
            )
            results = r.results
            kernel.last_run = r
        full[hidx] = _gather(results, len(hidx))
    return full
